# revision 1
# baseline (speedup 1.0000x reference)
"""BitLinear (ternary weight) inference kernel for Trainium2, 8-core SPMD.

Full-input contract: kernel(**inputs) takes the complete tensors and returns
the complete output. The batch dim (B=8) is sharded 1:1 onto the 8
NeuronCores; each core computes y[b] = x[b] @ (w_q * 2^s_exp)^T + bias as a
2048^3 matmul (fp16 x, fp8 w, fp32 PSUM accumulation).

Host prep (cheap, O(bytes)): fold the power-of-two per-channel scale into
the ternary weights — values +-2^s / 0 are EXACT in fp8e4m3 — transpose
both operands into the PE's contraction-major [K, ...] layout, cast x to
fp16 (the only lossy step, ~2^-11 relative), broadcast bias to [128, OUT].

Device schedule (PE-bound; ~245us/core vs 218.5us matmul streaming floor):
  - Mixed-dtype matmuls: stationary x-tile fp16 [128,128], moving w fp8
    [128,512], one PSUM bank each, K accumulated 128 rows per step.
  - The first 6 row tiles run k-chunks 0..3 as soon as ~2 MiB of input has
    landed (pass A), parking partial sums in SBUF; the remaining k-chunks
    are added later (accum pass) interleaved with full-k single-pass tiles,
    so the PE never waits on the 12.6 MiB input stream.
  - Inputs on the Sync HWDGE ring, output stores on the Scalar HWDGE ring,
    epilogue (psum + bias / + partial -> SBUF) on the Vector engine, and a
    short dummy-matmul burst pre-warms the PE HAM clock gate.
"""
import os

import ml_dtypes
import numpy as np

B, T, IN, OUT = 8, 2048, 2048, 2048
P = 128
NCORES = 8
NF = 512        # matmul free dim (one PSUM bank of fp32)
KA = 4          # k-chunks in pass A (first-pass dependency set = KA MiB won't gate PE)

last_exec_time_ns = None
_CACHE = {}


def _install_prof_shim():
    """Make antenv.axon_hooks importable so trace=True works under axon."""
    import sys
    import types

    if "antenv.axon_hooks" in sys.modules:
        return
    try:
        from trn_agent_boot.trn_boot import _ntff_profile_via_ctypes
    except ImportError:
        return
    hook = _ntff_profile_via_ctypes("/opt/axon/libaxon_pjrt.so")
    mod = types.ModuleType("antenv.axon_hooks")
    mod.get_axon_ntff_profile_hook = lambda: hook
    mod.set_axon_ntff_profile_hook = lambda h: None
    sys.modules["antenv.axon_hooks"] = mod


def _build():
    import concourse.bacc as bacc
    import concourse.mybir as mybir
    from concourse.tile import TileContext

    nc = bacc.Bacc()
    x = nc.dram_tensor("x", (IN, T), mybir.dt.float16, kind="ExternalInput")
    w = nc.dram_tensor("w", (IN, OUT), mybir.dt.float8e4, kind="ExternalInput")
    bias = nc.dram_tensor("bias", (P, OUT), mybir.dt.float32, kind="ExternalInput")
    y = nc.dram_tensor("y", (T, OUT), mybir.dt.float32, kind="ExternalOutput")

    KT = IN // P    # contraction chunks
    TT = T // P     # output row tiles
    OC = OUT // NF  # psum banks per row tile

    HOUT = OUT // 2  # two psum tiles (2 banks each) per row tile

    with TileContext(nc) as tc:
        with tc.tile_pool(name="wp", bufs=1) as wp, \
             tc.tile_pool(name="xp", bufs=1) as xp, \
             tc.tile_pool(name="bp", bufs=1) as bp, \
             tc.tile_pool(name="op", bufs=4) as op_, \
             tc.tile_pool(name="ptp", bufs=1) as ptp, \
             tc.tile_pool(name="pp", bufs=4, space="PSUM") as pp:

            # Interleave w/x chunk loads k-wise so pass A's working set
            # (k < KA) lands first and the PE can start after ~2 MiB.
            # Later chunks load pairwise (>=1 MiB DMAs for efficiency).
            w_tiles = [None] * KT
            xT_tiles = [None] * KT
            bias_t = bp.tile([P, OUT], mybir.dt.float32, tag="bias")
            x3 = x.rearrange("(ko p) t -> p ko t", p=P)
            w3 = w.rearrange("(ko p) o -> p ko o", p=P)

            # HAM pre-warm: a short burst of dummy matmuls on a scratch tile
            # while the first loads are in flight, so the PE clock-gate is
            # near 8/8 when the real matmuls start. Uses one "ps" slot
            # briefly (released well before pass A needs its 4th buffer).
            warm_sb = bp.tile([P, NF], mybir.dt.float16, tag="warm")
            nc.gpsimd.memset(warm_sb, 0.0)
            warm_ps = pp.tile([P, HOUT], mybir.dt.float32, tag="ps",
                              name="warmps")
            for i in range(6):
                nc.tensor.matmul(warm_ps[:, :NF], warm_sb[:, :P], warm_sb,
                                 start=(i == 0), stop=(i == 5))

            HT = T // 2
            for k in range(KA):
                wt = wp.tile([P, OUT], mybir.dt.float8e4, tag=f"w{k}")
                xt = xp.tile([P, T], mybir.dt.float16, tag=f"x{k}")
                nc.sync.dma_start(wt, w[k * P:(k + 1) * P, :])
                nc.sync.dma_start(xt[:, :HT], x[k * P:(k + 1) * P, :HT])
                w_tiles[k] = wt
                xT_tiles[k] = xt
            nc.sync.dma_start(bias_t, bias[:, :])
            for k in range(KA, KT, 2):
                wt2 = wp.tile([P, 2, OUT], mybir.dt.float8e4, tag=f"w{k}")
                nc.sync.dma_start(wt2, w3[:, k:k + 2, :])
                w_tiles[k] = wt2[:, 0]
                w_tiles[k + 1] = wt2[:, 1]
                xt2 = xp.tile([P, 2, T], mybir.dt.float16, tag=f"x{k}")
                nc.sync.dma_start(xt2, x3[:, k:k + 2, :])
                xT_tiles[k] = xt2[:, 0]
                xT_tiles[k + 1] = xt2[:, 1]
            # deferred: t>=1024 halves of the pass-A x chunks are only read
            # by single-pass row tiles 8+, which run ~50us after this lands
            for k in range(KA):
                nc.sync.dma_start(xT_tiles[k][:, HT:], x[k * P:(k + 1) * P, HT:])

            TSPLIT = 6       # row tiles 0..TSPLIT-1 two-pass (partials in SBUF)

            partial_tiles = [
                ptp.tile([P, OUT], mybir.dt.float32, tag=f"pt{j}", name=f"pt{j}")
                for j in range(TSPLIT)
            ]

            def do_tiles(tt_range, k_lo, k_hi, mode):
                # mode: "partial" = bias add into SBUF partial (no store),
                #       "accum" = add SBUF partial + store,
                #       "single" = bias add + store
                for tt in tt_range:
                    pss = [pp.tile([P, HOUT], mybir.dt.float32, tag="ps",
                                   name=f"ps{h}") for h in range(2)]
                    for k in range(k_lo, k_hi):
                        lhsT = xT_tiles[k][:, tt * P:(tt + 1) * P]
                        for oc in range(OC):
                            ps = pss[oc // 2]
                            lo = (oc % 2) * NF
                            nc.tensor.matmul(
                                ps[:, lo:lo + NF],
                                lhsT,
                                w_tiles[k][:, oc * NF:(oc + 1) * NF],
                                start=(k == k_lo),
                                stop=(k == k_hi - 1),
                            )
                    if mode == "partial":
                        ot = partial_tiles[tt]
                    else:
                        ot = op_.tile([P, OUT], mybir.dt.float32, tag="out")
                    if tt == TT - 1:
                        # last tile: chunk epilogue+store so the store of
                        # chunk q overlaps the add of chunk q+1 (short tail)
                        for q in range(OC):
                            sl = slice(q * NF, (q + 1) * NF)
                            psl = slice((q % 2) * NF, (q % 2) * NF + NF)
                            nc.vector.tensor_add(ot[:, sl], pss[q // 2][:, psl],
                                                 bias_t[:, sl])
                            eng = nc.scalar if q % 2 == 0 else nc.sync
                            eng.dma_start(y[tt * P:(tt + 1) * P, sl],
                                          ot[:, sl])
                        continue
                    for h in range(2):
                        sl = slice(h * HOUT, (h + 1) * HOUT)
                        if mode == "accum":
                            nc.vector.tensor_add(ot[:, sl], pss[h],
                                                 partial_tiles[tt][:, sl])
                        else:
                            nc.vector.tensor_add(ot[:, sl], pss[h], bias_t[:, sl])
                    if mode != "partial":
                        nc.scalar.dma_start(y[tt * P:(tt + 1) * P, :], ot)

            do_tiles(range(TSPLIT), 0, KA, "partial")
            # Interleave accum and single-pass tiles so the PE always has
            # runnable chunks while the tail of the input load streams in.
            for j in range(TT - TSPLIT):
                if j < TSPLIT:
                    do_tiles([j], KA, KT, "accum")
                do_tiles([TSPLIT + j], 0, KT, "single")

    nc.compile()
    return nc


def kernel(x, w_q, s_exp, bias):
    global last_exec_time_ns
    from concourse.bass_utils import run_bass_kernel_spmd

    x = np.asarray(x)
    w_q = np.asarray(w_q)
    s_exp = np.asarray(s_exp)
    bias = np.asarray(bias, dtype=np.float32)
    assert x.shape == (B, T, IN) and w_q.shape == (OUT, IN)

    # Fold the power-of-two per-output-channel scale into the ternary
    # weights: values are +-2^s or 0 with s in [-8, 0], exact in fp8e4m3
    # (2^-8 and 2^-9 are exact subnormals).
    scale = np.exp2(s_exp.astype(np.float32))
    w_scaled_t = (w_q.astype(np.float32) * scale[:, None]).T
    w_fp8 = np.ascontiguousarray(w_scaled_t).astype(ml_dtypes.float8_e4m3fn)
    if not np.array_equal(w_fp8.astype(np.float32), w_scaled_t):
        import warnings
        warnings.warn("scaled ternary weights not exact in fp8e4m3; "
                      "proceeding with rounded weights")
    bias_bcast = np.ascontiguousarray(
        np.broadcast_to(bias.astype(np.float32), (P, OUT)))
    # Contraction-major layout for the PE: x^T[b] = [IN, T], fp16.
    xT_f16 = np.ascontiguousarray(
        x.astype(np.float16).transpose(0, 2, 1))

    nc = _CACHE.get("nc")
    if nc is None:
        nc = _CACHE["nc"] = _build()

    in_maps = [
        {"x": xT_f16[b], "w": w_fp8, "bias": bias_bcast} for b in range(B)
    ]

    trace = bool(int(os.environ.get("BITLIN_TRACE", "0")))
    if trace:
        _install_prof_shim()
    res = run_bass_kernel_spmd(nc, in_maps, list(range(NCORES)), trace=trace)
    last_exec_time_ns = res.exec_time_ns

    out = np.stack([res.results[b]["y"] for b in range(B)], axis=0)
    return out.astype(np.float32, copy=False)



# revision 2
# speedup vs baseline: 1.4249x; 1.4249x over previous
"""BitLinear (ternary weight) inference kernel for Trainium2, 8-core SPMD.

Full-input contract: kernel(**inputs) takes the complete tensors and returns
the complete output. The batch dim (B=8) is sharded 1:1 onto the 8
NeuronCores; each core computes y[b] = x[b] @ (w_q * 2^s_exp)^T + bias as a
2048^3 matmul.

Split-precision scheme (the accuracy gate is max|err| / absmax(expected),
and both error and signal in column o scale with 2^s_exp[o]):
  - Output columns are permuted by s_exp descending. The top N16=512
    columns (all s=0/-1) run on an fp16(x) x fp8(w) path at bf16 rate.
  - The remaining 1536 columns run fp8(x) x fp8(w) with
    perf_mode=DoubleRow (K=256 per instruction, ~1.8x bf16 rate); their
    fp8-quantization error is scaled down by 2^s_exp <= 1/4, far below
    the gate. Measured on the reference data: ~1.3 abs vs 4.4 allowed.
  - Weights +-2^s / 0 are EXACT in fp8e4m3 (subnormals to 2^-9), so the
    only error sources are x quantization (fp16 / fp8) and the fp16
    output store (~2^-11).

Host prep (cheap, O(bytes), untimed): quantize + transpose x into
t-major fp16 tiles and k-pair-interleaved fp8 tiles, gather/fold the
weight columns, broadcast bias. All device DMAs are contiguous
[128 x multi-KiB-line] transfers.

Device schedule per core (PE-bound; ~147us ideal vs 218.5us fp16 floor):
  - Row tile t (128 rows): fp16 group = 16 matmuls [128k,128t]x[128k,512]
    into 1 PSUM bank; fp8 group = 8 k-pair DoubleRow matmuls x 3 chunks
    [128,2,128]x[128,2,512] into 3 banks. 4 banks per row tile, 8 total.
  - The first HEAD row tiles run fp16-only (x16 streams t-major, 0.5 MiB
    per tile, so the PE starts ~2us in) while the fp8/w stream (8 MiB)
    lands; then fp8 and remaining fp16 groups interleave.
  - Epilogue per bank on Vector (psum + bias -> fp16 SBUF), stores on the
    Scalar ring, x16 loads on the GpSimd ring, fp8/w loads on Sync.
"""
import os

import ml_dtypes
import numpy as np

B, T, IN, OUT = 8, 2048, 2048, 2048
P = 128
NCORES = 8
NF = 512          # psum bank width (fp32), matmul chunk
N16 = 512         # columns on the fp16 path (top s_exp)
N8 = OUT - N16    # columns on the fp8 DoubleRow path
KT = IN // P      # 16 k-chunks
KP = IN // (2 * P)  # 8 k-pairs
TT = T // P       # 16 row tiles
C8 = N8 // NF     # 3 fp8 chunks per row tile
HEAD = 10         # fp16-only row tiles before the first fp8 group

last_exec_time_ns = None
_CACHE = {}


def _install_prof_shim():
    """Make antenv.axon_hooks importable so trace=True works under axon."""
    import sys
    import types

    if "antenv.axon_hooks" in sys.modules:
        return
    try:
        from trn_agent_boot.trn_boot import _ntff_profile_via_ctypes
    except ImportError:
        return
    hook = _ntff_profile_via_ctypes("/opt/axon/libaxon_pjrt.so")
    mod = types.ModuleType("antenv.axon_hooks")
    mod.get_axon_ntff_profile_hook = lambda: hook
    mod.set_axon_ntff_profile_hook = lambda h: None
    sys.modules["antenv.axon_hooks"] = mod


def _build():
    import concourse.bacc as bacc
    import concourse.mybir as mybir
    from concourse.tile import TileContext

    DR = mybir.MatmulPerfMode.DoubleRow

    nc = bacc.Bacc()
    # t-major fp16 x: x16[tt, p, ko, m] = x[tt*128+m, ko*128+p]
    x16 = nc.dram_tensor("x16", (TT, P, KT, P), mybir.dt.float16,
                         kind="ExternalInput")
    # k-pair-interleaved fp8 x: x8[kp, p, i, t] = fp8(x)[t, kp*256+i*128+p]
    x8 = nc.dram_tensor("x8", (KP, P, 2, T), mybir.dt.float8e4,
                        kind="ExternalInput")
    # fp16-path weights (folded scale, permuted cols): w16[p, k, o]
    w16 = nc.dram_tensor("w16", (P, KT, N16), mybir.dt.float8e4,
                         kind="ExternalInput")
    # fp8-path weights, k-pair interleaved: w8[kp, p, i, o]
    w8 = nc.dram_tensor("w8", (KP, P, 2, N8), mybir.dt.float8e4,
                        kind="ExternalInput")
    bias = nc.dram_tensor("bias", (P, OUT), mybir.dt.float32,
                          kind="ExternalInput")
    y = nc.dram_tensor("y", (T, OUT), mybir.dt.float16, kind="ExternalOutput")

    with TileContext(nc) as tc:
        with tc.tile_pool(name="x16p", bufs=1) as x16p, \
             tc.tile_pool(name="x8p", bufs=1) as x8p, \
             tc.tile_pool(name="wp", bufs=1) as wp, \
             tc.tile_pool(name="bp", bufs=1) as bp, \
             tc.tile_pool(name="op", bufs=8) as op_, \
             tc.tile_pool(name="pp", bufs=8, space="PSUM") as pp:

            # HAM pre-warm: dummy matmuls while the first loads land so the
            # PE clock gate is near 8/8 when real work starts.
            warm_sb = bp.tile([P, NF], mybir.dt.float16, tag="warm")
            nc.gpsimd.memset(warm_sb, 0.0)
            warm_ps = pp.tile([P, NF], mybir.dt.float32, tag="ps",
                              name="warmps")
            for i in range(6):
                nc.tensor.matmul(warm_ps, warm_sb[:, :P], warm_sb,
                                 start=(i == 0), stop=(i == 5))

            # --- input loads ---
            w16_sb = wp.tile([P, KT, N16], mybir.dt.float8e4, tag="w16")
            nc.sync.dma_start(w16_sb, w16[:, :, :])

            x16_sb = [None] * TT
            # x16 t-tiles on the gpsimd ring, split in 4 so the first
            # matmul's dependency is 128 KiB, not 512 KiB.
            for tt in range(2):
                xt = x16p.tile([P, KT, P], mybir.dt.float16, tag=f"x16_{tt}")
                for q in range(0, KT, 4):
                    nc.gpsimd.dma_start(xt[:, q:q + 4, :],
                                        x16[tt, :, q:q + 4, :])
                x16_sb[tt] = xt
            bias_sb = bp.tile([P, OUT], mybir.dt.float32, tag="bias")
            nc.gpsimd.dma_start(bias_sb, bias[:, :])
            for tt in range(2, TT):
                xt = x16p.tile([P, KT, P], mybir.dt.float16, tag=f"x16_{tt}")
                nc.gpsimd.dma_start(xt, x16[tt, :, :, :])
                x16_sb[tt] = xt

            x8_sb = [None] * KP
            w8_sb = [None] * KP
            for kp in range(KP):
                x8t = x8p.tile([P, 2, T], mybir.dt.float8e4, tag=f"x8_{kp}")
                nc.sync.dma_start(x8t, x8[kp, :, :, :])
                x8_sb[kp] = x8t
                w8t = wp.tile([P, 2, N8], mybir.dt.float8e4, tag=f"w8_{kp}")
                nc.sync.dma_start(w8t, w8[kp, :, :, :])
                w8_sb[kp] = w8t

            # --- compute groups ---
            def f16_group(tt):
                ps = pp.tile([P, NF], mybir.dt.float32, tag="ps",
                             name=f"f16ps{tt}")
                xt = x16_sb[tt]
                for k in range(KT):
                    nc.tensor.matmul(ps, xt[:, k, :], w16_sb[:, k, :],
                                     start=(k == 0), stop=(k == KT - 1))
                ot = op_.tile([P, NF], mybir.dt.float16, tag="out")
                nc.vector.tensor_add(ot, ps, bias_sb[:, :N16])
                nc.scalar.dma_start(y[tt * P:(tt + 1) * P, :N16], ot)

            def f8_group(tt):
                pss = [pp.tile([P, NF], mybir.dt.float32, tag="ps",
                               name=f"f8ps{tt}_{c}") for c in range(C8)]
                for kp in range(KP):
                    lhsT = x8_sb[kp][:, :, tt * P:(tt + 1) * P]
                    for c in range(C8):
                        nc.tensor.matmul(
                            pss[c], lhsT,
                            w8_sb[kp][:, :, c * NF:(c + 1) * NF],
                            start=(kp == 0), stop=(kp == KP - 1),
                            perf_mode=DR)
                for c in range(C8):
                    ot = op_.tile([P, NF], mybir.dt.float16, tag="out")
                    sl = slice(N16 + c * NF, N16 + (c + 1) * NF)
                    nc.vector.tensor_add(ot, pss[c], bias_sb[:, sl])
                    nc.scalar.dma_start(y[tt * P:(tt + 1) * P, sl], ot)

            # fp16 head start while the fp8/w stream lands, then interleave.
            for tt in range(HEAD):
                f16_group(tt)
            t8 = 0
            for tt in range(HEAD, TT):
                f8_group(t8)
                t8 += 1
                f16_group(tt)
            while t8 < TT:
                f8_group(t8)
                t8 += 1

    nc.compile()
    return nc


def kernel(x, w_q, s_exp, bias):
    global last_exec_time_ns
    from concourse.bass_utils import run_bass_kernel_spmd

    x = np.asarray(x)
    w_q = np.asarray(w_q)
    s_exp = np.asarray(s_exp)
    bias = np.asarray(bias, dtype=np.float32)
    assert x.shape == (B, T, IN) and w_q.shape == (OUT, IN)

    # Fold the power-of-two per-output-channel scale into the ternary
    # weights: values are +-2^s or 0 with s in [-8, 0], exact in fp8e4m3.
    scale = np.exp2(s_exp.astype(np.float32))
    w_scaled = w_q.astype(np.float32) * scale[:, None]  # [OUT, IN]

    # Columns sorted by s_exp descending: first N16 -> fp16 path.
    perm = np.argsort(-s_exp.astype(np.int64), kind="stable")
    wp_t = np.ascontiguousarray(w_scaled[perm].T)  # [IN, OUT] permuted cols
    w_fp8 = wp_t.astype(ml_dtypes.float8_e4m3fn)
    if not np.array_equal(w_fp8.astype(np.float32), wp_t):
        import warnings
        warnings.warn("scaled ternary weights not exact in fp8e4m3; "
                      "proceeding with rounded weights")

    # w16[p, k, o] = w[k*128+p, o<N16]
    w16 = np.ascontiguousarray(
        w_fp8[:, :N16].reshape(KT, P, N16).transpose(1, 0, 2))
    # w8[kp, p, i, o] = w[kp*256+i*128+p, N16+o]
    w8 = np.ascontiguousarray(
        w_fp8[:, N16:].reshape(KP, 2, P, N8).transpose(0, 2, 1, 3))
    bias_p = np.ascontiguousarray(
        np.broadcast_to(bias[perm].astype(np.float32), (P, OUT)))

    x16_t = np.empty((B, TT, P, KT, P), dtype=np.float16)
    x8_t = np.empty((B, KP, P, 2, T), dtype=ml_dtypes.float8_e4m3fn)
    for b in range(B):
        xb16 = x[b].astype(np.float16)  # [T, IN]
        # x16[tt, p, ko, m] = x[tt*128+m, ko*128+p]
        x16_t[b] = xb16.reshape(TT, P, KT, P).transpose(0, 3, 2, 1)
        xq = x[b].astype(ml_dtypes.float8_e4m3fn)  # [T, IN]
        # x8[kp, p, i, t] = xq[t, kp*256+i*128+p]
        x8_t[b] = np.ascontiguousarray(xq.T).reshape(
            KP, 2, P, T).transpose(0, 2, 1, 3)

    nc = _CACHE.get("nc")
    if nc is None:
        nc = _CACHE["nc"] = _build()

    in_maps = [
        {"x16": x16_t[b], "x8": x8_t[b], "w16": w16, "w8": w8,
         "bias": bias_p} for b in range(B)
    ]

    trace = bool(int(os.environ.get("BITLIN_TRACE", "0")))
    if trace:
        _install_prof_shim()
    res = run_bass_kernel_spmd(nc, in_maps, list(range(NCORES)), trace=trace)
    last_exec_time_ns = res.exec_time_ns

    out = np.empty((B, T, OUT), dtype=np.float32)
    inv = np.empty_like(perm)
    inv[perm] = np.arange(OUT)
    for b in range(B):
        out[b] = res.results[b]["y"].astype(np.float32)[:, inv]
    return out
